# revision 22
# baseline (speedup 1.0000x reference)
"""Multi-head attention block (q/k/v projections + softmax attention +
out-projection) distributed over 8 TRN2 NeuronCores.

Sharding: core c handles batch b = c//2 and query rows [h*1024, (h+1)*1024),
h = c%2. Each core keeps the full kv of its batch (kv projections are
recomputed per query-half) so no inter-core collective is needed; the full
output is assembled host-side from disjoint shards.

Host-side prep (not on the HW critical path): q/kv are cast to bf16 and
pre-transposed to [model_dim, seq]; weights are cast to bf16. The device
then DMAs operands straight into their compute layouts — no on-device
casts, no PE transposes, and half the HBM input bytes.

Per-core dataflow:
  projections (bf16 matmuls, fp32 PSUM):
       qhT/khT [inner, seq] transposed layout bf16, vh [seq_k, head*(64+1)]
       natural bf16 with a ones column (P@[V|1] then yields the softmax
       denominator for free)
  attention per head-pair (two heads row-packed in the 128-row PE array via
  tile_position, contraction dim HEAD_DIM=64):
       scores S^T[k, q] on fp32 PSUM -> exp(s/8) fused on ScalarE -> bf16
       -> PV [65, q] PSUM accumulation over the 16 k tiles
       -> denominator row -> Kc=1 ones-matmul broadcast -> fast reciprocal
       -> multiply -> attnT bf16 (no max-subtraction needed: logits are
       O(5) so exp is safe in fp32)
  out-projection (bf16): per-pair partial matmuls accumulated into an SBUF
  fp32 buffer (bias folded into the first partial), streamed into the
  attention phase as PE filler.

The attention k-loop is software-pipelined depth-2 ([scores(k) | filler |
PV(k-2)]) and the last PV + normalization of each block carry into the next
block's prologue, so neither the TensorEngine nor ScalarE stalls at block
boundaries; leftover projection / out-projection work rides in the bubbles
as filler. Only kvT/qT columns 0:512 plus wk/wq are loaded before attention
starts; the rest streams in behind it. The last block's normalization is
split in half so the final out-projection rows and output DMA pipeline
with it.
"""

import sys

sys.path.insert(0, "/opt/trn_rl_repo")

import numpy as np

B, NQ_FULL, NK = 4, 2048, 2048
NQ = 1024          # per-core query rows
DQ, DKV = 512, 768
HEADS, DH = 8, 64
INNER = 512
DA = DH + 1        # head dim + ones column
N_CORES = 8

_cache = {}


def _build():
    import concourse.bass as bass
    import concourse.tile as tile
    from concourse import bacc, mybir

    F32 = mybir.dt.float32
    F32R = mybir.dt.float32r
    BF16 = mybir.dt.bfloat16
    EXP = mybir.ActivationFunctionType.Exp

    nc = bacc.Bacc("TRN2", target_bir_lowering=False, debug=False,
                   enable_asserts=True, num_devices=N_CORES)

    qT_d = nc.dram_tensor("qT", [DQ, NQ], BF16, kind="ExternalInput").ap()
    kvT_d = nc.dram_tensor("kvT", [DKV, NK], BF16, kind="ExternalInput").ap()
    wq_d = nc.dram_tensor("Wq", [DQ, INNER], BF16, kind="ExternalInput").ap()
    wk_d = nc.dram_tensor("Wk", [DKV, INNER], BF16, kind="ExternalInput").ap()
    wv_d = nc.dram_tensor("Wv", [DKV, INNER], BF16, kind="ExternalInput").ap()
    wo_d = nc.dram_tensor("Wo", [INNER, DQ], BF16, kind="ExternalInput").ap()
    bo_d = nc.dram_tensor("bo", [DQ], F32, kind="ExternalInput").ap()
    out_d = nc.dram_tensor("out", [NQ, DQ], F32, kind="ExternalOutput").ap()
    # vh exchange bounce: each core contributes its local half (heads 0-3,
    # duplicated into both rank chunks); pairwise ReduceScatter(add) returns
    # local+peer identically on both cores; subtracting the local copy
    # recovers the peer's half with no rank-dependent addressing.
    xin_d = nc.dram_tensor("xin", [2, 128, 16 * 256], mybir.dt.bfloat16,
                           kind="Internal").ap()
    xout_d = nc.dram_tensor("xout", [128, 16 * 256], mybir.dt.bfloat16,
                            kind="Internal").ap()

    MT_Q = DQ // 128      # 4
    MT_KV = DKV // 128    # 6
    IT = INNER // 128     # 4 inner tiles (= head pairs)
    KT = NK // 128        # 16
    QB = NQ // 512        # 2
    NT = NQ // 128        # 8 output row tiles
    PAIRS = HEADS // 2    # 4

    with tile.TileContext(nc) as tc:
        with (
            tc.tile_pool(name="consts", bufs=1) as consts,
            tc.tile_pool(name="wpool", bufs=1) as wpool,
            tc.tile_pool(name="xT", bufs=1) as xT_pool,
            tc.tile_pool(name="proj", bufs=1) as proj_pool,
            tc.tile_pool(name="attnT", bufs=1) as attnT_pool,
            tc.tile_pool(name="oacc", bufs=1) as oacc_pool,
            tc.tile_pool(name="exps", bufs=3) as exps_pool,
            tc.tile_pool(name="norm", bufs=1) as norm_pool,
            tc.tile_pool(name="outs", bufs=2) as outs_pool,
            tc.tile_pool(name="mm", bufs=2, space="PSUM") as ps_mm,
            tc.tile_pool(name="sc", bufs=2, space="PSUM") as ps_sc,
            tc.tile_pool(name="pv", bufs=2, space="PSUM") as ps_pv,
        ):
            wk_b = wpool.tile([128, MT_KV, 512], BF16, tag="wk")
            wq_b = wpool.tile([128, MT_Q, 512], BF16, tag="wq")
            wv_b = wpool.tile([128, MT_KV, 512], BF16, tag="wv")
            wo_b = wpool.tile([128, IT, 512], BF16, tag="wo")
            kvT = [xT_pool.tile([128, NK], BF16, tag=f"kvT{mt}", name=f"kvT{mt}")
                   for mt in range(MT_KV)]
            qT = [xT_pool.tile([128, NQ], BF16, tag=f"qT{mt}", name=f"qT{mt}")
                  for mt in range(MT_Q)]

            # ---- DMA issue order: the pre-attention critical prefix first
            # (kvT/qT columns 0:512 + wk/wq/wv), the rest streams in behind
            # the first attention block.
            nc.sync.dma_start(wk_b[:],
                              wk_d.rearrange("(t p) i -> p t i", p=128))
            nc.sync.dma_start(wq_b[:],
                              wq_d.rearrange("(t p) i -> p t i", p=128))
            for mt in range(MT_KV):
                nc.sync.dma_start(kvT[mt][:, 0:512],
                                  kvT_d[mt * 128:(mt + 1) * 128, 0:512])
            for mt in range(MT_Q):
                nc.sync.dma_start(qT[mt][:, 0:512],
                                  qT_d[mt * 128:(mt + 1) * 128, 0:512])
            nc.sync.dma_start(wv_b[:],
                              wv_d.rearrange("(t p) i -> p t i", p=128))
            for mt in range(MT_KV):
                nc.sync.dma_start(kvT[mt][:, 512:NK],
                                  kvT_d[mt * 128:(mt + 1) * 128, 512:NK])
            for mt in range(MT_Q):
                nc.sync.dma_start(qT[mt][:, 512:NQ],
                                  qT_d[mt * 128:(mt + 1) * 128, 512:NQ])
            nc.sync.dma_start(wo_b[:],
                              wo_d.rearrange("(t p) i -> p t i", p=128))

            # ---- constants ----
            ones1f = consts.tile([1, 64], F32)
            nc.vector.memset(ones1f[:], 1.0)
            ones1 = consts.tile([1, 64], F32R)
            nc.vector.tensor_copy(ones1[:], ones1f[:])
            ones8 = consts.tile([128, 8, 1], BF16)
            ones8f = consts.tile([128, 8, 1], F32)
            nc.vector.memset(ones8f[:], 1.0)
            nc.vector.tensor_copy(ones8[:], ones8f[:])
            warm = consts.tile([1, 64], mybir.dt.bfloat16)
            nc.scalar.activation(warm[:], ones1f[:], EXP, scale=0.125)
            bo_b = consts.tile([128, DQ], F32)
            nc.gpsimd.dma_start(
                out=bo_b[:],
                in_=bass.AP(tensor=bo_d.tensor, offset=bo_d.offset,
                            ap=[[0, 128]] + list(bo_d.ap)),
            )

            # ---- projection outputs / accumulators ----
            qhT = [proj_pool.tile([128, NQ], BF16, tag=f"qhT{i}", name=f"qhT{i}")
                   for i in range(IT)]
            khT = [proj_pool.tile([128, NK], BF16, tag=f"khT{i}", name=f"khT{i}")
                   for i in range(IT)]
            vh = [proj_pool.tile([128, HEADS, DA], BF16, tag=f"vh{k}", name=f"vh{k}")
                  for k in range(KT)]
            attnT = [attnT_pool.tile([128, NQ], BF16, tag=f"at{i}", name=f"at{i}")
                     for i in range(IT)]
            oacc = [oacc_pool.tile([128, DQ], F32, tag=f"oa{nt}", name=f"oa{nt}")
                    for nt in range(NT)]

            def emit_khT(it, nb):
                pp = ps_mm.tile([128, 512], F32, tag="mm", name="pp")
                for mt in range(MT_KV):
                    nc.tensor.matmul(
                        pp[:], wk_b[:, mt, it * 128:(it + 1) * 128],
                        kvT[mt][:, nb * 512:(nb + 1) * 512],
                        start=(mt == 0), stop=(mt == MT_KV - 1))
                nc.vector.tensor_copy(khT[it][:, nb * 512:(nb + 1) * 512], pp[:])

            def emit_qhT(it, nb):
                pp = ps_mm.tile([128, 512], F32, tag="mm", name="pp")
                for mt in range(MT_Q):
                    nc.tensor.matmul(
                        pp[:], wq_b[:, mt, it * 128:(it + 1) * 128],
                        qT[mt][:, nb * 512:(nb + 1) * 512],
                        start=(mt == 0), stop=(mt == MT_Q - 1))
                nc.vector.tensor_copy(qhT[it][:, nb * 512:(nb + 1) * 512], pp[:])

            def emit_vh(kt):
                # local half only: kernel heads 0-3 (= this core's global
                # heads via the host-side weight permutation)
                pp = ps_mm.tile([128, 256], F32, tag="mm", name="pp",
                                padded_shape=[128, 512])
                for mt in range(MT_KV):
                    nc.tensor.matmul(
                        pp[:], kvT[mt][:, kt * 128:(kt + 1) * 128],
                        wv_b[:, mt, 0:256],
                        start=(mt == 0), stop=(mt == MT_KV - 1))
                nc.vector.tensor_copy(
                    vh[kt][:, 0:4, 0:DH],
                    pp[:].rearrange("p (h d) -> p h d", h=4))
                nc.vector.tensor_copy(vh[kt][:, :, DH:DA], ones8[:])

            def vh_send(kts):
                for kt in kts:
                    for c in (0, 1):
                        nc.sync.dma_start(
                            xin_d[c:c + 1, :, kt * 256:(kt + 1) * 256],
                            vh[kt][:, 0:4, 0:DH])

            def vh_exchange():
                nc.gpsimd.collective_compute(
                    "ReduceScatter", mybir.AluOpType.add,
                    replica_groups=[[0, 1], [2, 3], [4, 5], [6, 7]],
                    ins=[xin_d], outs=[xout_d])

            def vh_recv(kts):
                for kt in kts:
                    nc.sync.dma_start(vh[kt][:, 4:8, 0:DH],
                                      xout_d[:, kt * 256:(kt + 1) * 256])
                for kt in kts:
                    nc.vector.tensor_sub(vh[kt][:, 4:8, 0:DH],
                                         vh[kt][:, 4:8, 0:DH],
                                         vh[kt][:, 0:4, 0:DH])

            def emit_opart(t, nt):
                # out-projection partial for head pair t, row tile nt:
                # oacc[nt] (+)= attnT[t][:, ns].T @ Wo[pair t rows]
                ns = slice(nt * 128, (nt + 1) * 128)
                po = ps_mm.tile([128, 512], F32, tag="mm", name="po")
                nc.tensor.matmul(po[:], attnT[t][:, ns], wo_b[:, t, :],
                                 start=True, stop=True)
                if t == 0:
                    nc.vector.tensor_add(oacc[nt][:], po[:], bo_b[:])
                else:
                    nc.vector.tensor_add(oacc[nt][:], po[:], oacc[nt][:])

            def emit_final(nt):
                # last out-projection partial (pair 3) + bias-carrying
                # accumulator -> output row tile store
                ns = slice(nt * 128, (nt + 1) * 128)
                po = ps_mm.tile([128, 512], F32, tag="mm", name="po")
                nc.tensor.matmul(po[:], attnT[3][:, ns], wo_b[:, 3, :],
                                 start=True, stop=True)
                ot = outs_pool.tile([128, DQ], F32, tag="ot", name="ot")
                nc.vector.tensor_add(ot[:], po[:], oacc[nt][:])
                nc.sync.dma_start(out_d[ns, :], ot[:])

            # pre-attention minimum: pair-0 projections for the first 512 q
            # and first 4 k-tiles; everything else rides as filler.
            emit_khT(0, 0)
            emit_qhT(0, 0)

            # PE filler queues per (pair, q-block). khT(0, nb) is consumed by
            # scores(kt=4nb..) of block (0, 0); vh(kt) by PV(kt) two
            # iterations after scores(kt); opart(t, nt) needs norm(t, nt//4)
            # which runs at the start of the next block after that.
            fillers = {
                (0, 0): ([(lambda kt=kt: emit_vh(kt)) for kt in (0, 1, 2)]
                         + [lambda: emit_khT(0, 1)]
                         + [(lambda kt=kt: emit_vh(kt)) for kt in (3, 4)]
                         + [lambda: emit_khT(0, 2)]
                         + [(lambda kt=kt: emit_vh(kt)) for kt in (5, 6)]
                         + [lambda: emit_khT(0, 3)]
                         + [lambda: emit_qhT(0, 1)]
                         + [(lambda kt=kt: emit_vh(kt)) for kt in range(7, KT)]
                         + [lambda: vh_send(range(0, 8))]
                         + [lambda: (vh_send(range(8, KT)), vh_exchange())]),
                (0, 1): ([(lambda nb=nb: emit_khT(1, nb)) for nb in range(4)]
                         + [(lambda nb=nb: emit_qhT(1, nb)) for nb in range(QB)]),
                (1, 0): ([(lambda nt=nt: emit_opart(0, nt)) for nt in range(4)]
                         + [(lambda nb=nb: emit_khT(2, nb)) for nb in range(2)]),
                (1, 1): ([(lambda nb=nb: emit_khT(2, nb)) for nb in range(2, 4)]
                         + [(lambda nb=nb: emit_qhT(2, nb)) for nb in range(QB)]
                         + [(lambda nt=nt: emit_opart(0, nt)) for nt in range(4, 6)]
                         + [lambda: vh_recv(range(0, 8))]
                         + [lambda: vh_recv(range(8, KT))]),
                (2, 0): ([(lambda nt=nt: emit_opart(0, nt)) for nt in range(6, 8)]
                         + [(lambda nb=nb: emit_khT(3, nb)) for nb in range(2)]
                         + [(lambda nt=nt: emit_opart(1, nt)) for nt in range(2)]),
                (2, 1): ([(lambda nb=nb: emit_khT(3, nb)) for nb in range(2, 4)]
                         + [(lambda nb=nb: emit_qhT(3, nb)) for nb in range(QB)]
                         + [(lambda nt=nt: emit_opart(1, nt)) for nt in range(2, 4)]),
                (3, 0): ([(lambda nt=nt: emit_opart(1, nt)) for nt in range(4, 8)]
                         + [(lambda nt=nt: emit_opart(2, nt)) for nt in range(2)]),
                (3, 1): ([(lambda nt=nt: emit_opart(2, nt)) for nt in range(2, 8)]
                         + [(lambda nt=nt: emit_final(nt)) for nt in range(4)]),
            }

            carry = [None]

            def make_norm(t, qb, pvA, pvB):
                def emit():
                    qs = slice(qb * 512, (qb + 1) * 512)
                    dsb = norm_pool.tile([1, 1024], F32R, tag="nrm", name="dsb")
                    nc.vector.tensor_copy(dsb[0:1, 0:512], pvA[DH:DA, :])
                    nc.vector.tensor_copy(dsb[0:1, 512:1024], pvB[DH:DA, :])
                    dba = ps_mm.tile([64, 512], F32, tag="mm", name="dba")
                    dbb = ps_mm.tile([64, 512], F32, tag="mm", name="dbb")
                    rb = norm_pool.tile([64, 1024], F32, tag="nrm", name="rb")
                    nc.tensor.matmul(dba[:], ones1[:], dsb[0:1, 0:512],
                                     start=True, stop=True)
                    nc.vector.reciprocal_approx_fast(rb[:, 0:512], dba[:])
                    nc.vector.tensor_mul(attnT[t][0:64, qs],
                                         pvA[0:DH, :], rb[:, 0:512])
                    nc.tensor.matmul(dbb[:], ones1[:], dsb[0:1, 512:1024],
                                     start=True, stop=True)
                    nc.vector.reciprocal_approx_fast(rb[:, 512:1024], dbb[:])
                    nc.vector.tensor_mul(attnT[t][64:128, qs],
                                         pvB[0:DH, :], rb[:, 512:1024])
                return emit

            def norm_half_pre(t, qb, pvA, pvB, h):
                cs = slice(h * 256, (h + 1) * 256)
                dsb = norm_pool.tile([1, 512], F32R, tag="nrmd", name="dsb",
                                     bufs=2)
                nc.vector.tensor_copy(dsb[0:1, 0:256], pvA[DH:DA, cs])
                nc.vector.tensor_copy(dsb[0:1, 256:512], pvB[DH:DA, cs])
                return dsb

            def norm_half(t, qb, pvA, pvB, h, dsb):
                # normalize a 256-col half of the last block so the final
                # out-projection rows pipeline with the remaining half
                cs = slice(h * 256, (h + 1) * 256)
                qs = slice(qb * 512 + h * 256, qb * 512 + (h + 1) * 256)
                dba = ps_mm.tile([64, 256], F32, tag="mm", name="dba")
                dbb = ps_mm.tile([64, 256], F32, tag="mm", name="dbb")
                nc.tensor.matmul(dba[:], ones1[:], dsb[0:1, 0:256],
                                 start=True, stop=True)
                nc.tensor.matmul(dbb[:], ones1[:], dsb[0:1, 256:512],
                                 start=True, stop=True)
                rb = norm_pool.tile([64, 512], F32, tag="nrmh", name="rb")
                nc.vector.reciprocal_approx_fast(rb[:, 0:256], dba[:])
                nc.vector.reciprocal_approx_fast(rb[:, 256:512], dbb[:])
                nc.vector.tensor_mul(attnT[t][0:64, qs], pvA[0:DH, cs],
                                     rb[:, 0:256])
                nc.vector.tensor_mul(attnT[t][64:128, qs], pvB[0:DH, cs],
                                     rb[:, 256:512])

            # ---- attention, software-pipelined [scores(k+1) | filler | PV(k)]
            for t in range(PAIRS):
                hA, hB = 2 * t, 2 * t + 1
                for qb in range(QB):
                    qs = slice(qb * 512, (qb + 1) * 512)
                    todo = fillers[(t, qb)]
                    fi = 0

                    def emit_scores(kt):
                        ks = slice(kt * 128, (kt + 1) * 128)
                        sc = ps_sc.tile([128, 1024], F32, tag="sc", name="sc")
                        nc.tensor.matmul(
                            sc[:, 0:512],
                            khT[t][0:64, ks], qhT[t][0:64, qs],
                            start=True, stop=True, tile_position=(0, 0))
                        nc.tensor.matmul(
                            sc[:, 512:1024],
                            khT[t][64:128, ks], qhT[t][64:128, qs],
                            start=True, stop=True, tile_position=(64, 0))
                        ex = exps_pool.tile([128, 1024], BF16, tag="exp", name="ex")
                        nc.scalar.activation(ex[:], sc[:], EXP,
                                             scale=float(DH) ** -0.5)
                        return ex

                    # depth-2 software pipeline: PV(k) trails scores(k) by
                    # two iterations; the last two PVs + normalization of this
                    # block carry into the next block's prologue so ScalarE is
                    # never starved at block boundaries.
                    exq = [emit_scores(0)]
                    if carry[0]:
                        carry[0][0]()      # PV(14) of the previous block
                    exq.append(emit_scores(1))
                    if carry[0]:
                        carry[0][1]()      # PV(15) of the previous block
                        carry[0][2]()      # normalization (frees old pv tiles)
                        carry[0] = None
                    pvA = ps_pv.tile([DA, 512], F32, tag="pv", name="pvA")
                    pvB = ps_pv.tile([DA, 512], F32, tag="pv", name="pvB")

                    def mk_pv(kt, ex, pvA=pvA, pvB=pvB, hA=hA, hB=hB):
                        def emit():
                            nc.tensor.matmul(pvA[:], vh[kt][:, hA, :],
                                             ex[:, 0:512],
                                             start=(kt == 0), stop=(kt == KT - 1))
                            nc.tensor.matmul(pvB[:], vh[kt][:, hB, :],
                                             ex[:, 512:1024],
                                             start=(kt == 0), stop=(kt == KT - 1))
                        return emit

                    per_iter = 2 if (t, qb) in ((0, 0), (3, 1)) else 1
                    for kt in range(2, KT):
                        exq.append(emit_scores(kt))
                        for _ in range(per_iter):
                            if fi < len(todo) and (per_iter == 2 or kt % 2 == 0):
                                todo[fi]()
                                fi += 1
                        mk_pv(kt - 2, exq[kt - 2])()
                    mk_pv(KT - 2, exq[KT - 2])()
                    while fi < len(todo):
                        todo[fi]()
                        fi += 1
                    carry[0] = [mk_pv(KT - 1, exq[KT - 1]), lambda: None,
                                make_norm(t, qb, pvA, pvB), (pvA, pvB)]
            carry[0][0]()

            # ---- chunked tail: half-norms interleaved with final rows ----
            pvA_l, pvB_l = carry[0][3]
            dsb0 = norm_half_pre(3, 1, pvA_l, pvB_l, 0)
            dsb1 = norm_half_pre(3, 1, pvA_l, pvB_l, 1)
            norm_half(3, 1, pvA_l, pvB_l, 0, dsb0)
            emit_final(4)
            emit_final(5)
            norm_half(3, 1, pvA_l, pvB_l, 1, dsb1)
            emit_final(6)
            emit_final(7)

    nc.compile()
    return nc


def make_in_maps(q, kv, Wq, Wk, Wv, Wo, bo):
    """Host-side prep: bf16 casts + [dim, seq] transposes, per core."""
    import ml_dtypes
    bf16 = ml_dtypes.bfloat16

    wq_b = np.ascontiguousarray(np.asarray(Wq, np.float32).astype(bf16))
    wk_b = np.ascontiguousarray(np.asarray(Wk, np.float32).astype(bf16))
    wv_b = np.ascontiguousarray(np.asarray(Wv, np.float32).astype(bf16))
    wo_b = np.ascontiguousarray(np.asarray(Wo, np.float32).astype(bf16))
    bo_f = np.ascontiguousarray(np.asarray(bo, np.float32))
    q = np.asarray(q, np.float32)
    kv = np.asarray(kv, np.float32)

    kvT_b = [np.ascontiguousarray(kv[b].T.astype(bf16)) for b in range(B)]
    # head-permuted weight variants: core h of each pair sees its own 4
    # global heads as kernel heads 0-3 (inner axis rolled by 256 for h=1);
    # permuting Wq/Wk/Wv columns and Wo rows identically keeps the output
    # exact while letting each core compute only kernel heads 0-3 of vh.
    perm = np.r_[256:512, 0:256]
    wsets = [
        (wq_b, wk_b, wv_b, wo_b),
        (np.ascontiguousarray(wq_b[:, perm]),
         np.ascontiguousarray(wk_b[:, perm]),
         np.ascontiguousarray(wv_b[:, perm]),
         np.ascontiguousarray(wo_b[perm, :])),
    ]
    in_maps = []
    for c in range(N_CORES):
        b, h = c // 2, c % 2
        qT_b = np.ascontiguousarray(
            q[b, h * NQ:(h + 1) * NQ].T.astype(bf16))
        wq_c, wk_c, wv_c, wo_c = wsets[h]
        in_maps.append({
            "qT": qT_b, "kvT": kvT_b[b],
            "Wq": wq_c, "Wk": wk_c, "Wv": wv_c, "Wo": wo_c, "bo": bo_f,
        })
    return in_maps


def kernel(q, kv, Wq, Wk, Wv, Wo, bo):
    from concourse.bass_utils import run_bass_kernel_spmd

    if "nc" not in _cache:
        _cache["nc"] = _build()
    nc = _cache["nc"]

    in_maps = make_in_maps(q, kv, Wq, Wk, Wv, Wo, bo)
    res = run_bass_kernel_spmd(nc, in_maps, core_ids=list(range(N_CORES)))
    out = np.empty((B, NQ_FULL, DQ), dtype=np.float32)
    for c in range(N_CORES):
        b, h = c // 2, c % 2
        out[b, h * NQ:(h + 1) * NQ] = res.results[c]["out"]
    return out


# revision 23
# speedup vs baseline: 1.1886x; 1.1886x over previous
"""Multi-head attention block (q/k/v projections + softmax attention +
out-projection) distributed over 8 TRN2 NeuronCores.

Sharding: core c handles batch b = c//2 and query rows [h*1024, (h+1)*1024),
h = c%2. Each core keeps the full kv of its batch (kv projections are
recomputed per query-half) so no inter-core collective is needed; the full
output is assembled host-side from disjoint shards.

Host-side prep (not on the HW critical path): q/kv are cast to bf16 and
pre-transposed to [model_dim, seq]; weights are cast to bf16. The device
then DMAs operands straight into their compute layouts — no on-device
casts, no PE transposes, and half the HBM input bytes.

Per-core dataflow:
  projections (bf16 matmuls, fp32 PSUM):
       qhT/khT [inner, seq] transposed layout bf16, vh [seq_k, head*(64+1)]
       natural bf16 with a ones column (P@[V|1] then yields the softmax
       denominator for free)
  attention per head-pair (two heads row-packed in the 128-row PE array via
  tile_position, contraction dim HEAD_DIM=64):
       scores S^T[k, q] on fp32 PSUM -> exp(s/8) fused on ScalarE -> bf16
       -> PV [65, q] PSUM accumulation over the 16 k tiles
       -> denominator row -> Kc=1 ones-matmul broadcast -> fast reciprocal
       -> multiply -> attnT bf16 (no max-subtraction needed: logits are
       O(5) so exp is safe in fp32)
  out-projection (bf16): per-pair partial matmuls accumulated into an SBUF
  fp32 buffer (bias folded into the first partial), streamed into the
  attention phase as PE filler.

The attention k-loop is software-pipelined depth-2 ([scores(k) | filler |
PV(k-2)]) and the last PV + normalization of each block carry into the next
block's prologue, so neither the TensorEngine nor ScalarE stalls at block
boundaries; leftover projection / out-projection work rides in the bubbles
as filler. Only kvT/qT columns 0:512 plus wk/wq are loaded before attention
starts; the rest streams in behind it. The last block's normalization is
split in half so the final out-projection rows and output DMA pipeline
with it.
"""

import sys

sys.path.insert(0, "/opt/trn_rl_repo")

import numpy as np

B, NQ_FULL, NK = 4, 2048, 2048
NQ = 1024          # per-core query rows
DQ, DKV = 512, 768
HEADS, DH = 8, 64
INNER = 512
DA = DH + 1        # head dim + ones column
N_CORES = 8

_cache = {}


def _build():
    import concourse.bass as bass
    import concourse.tile as tile
    from concourse import bacc, mybir

    F32 = mybir.dt.float32
    F32R = mybir.dt.float32r
    BF16 = mybir.dt.bfloat16
    EXP = mybir.ActivationFunctionType.Exp

    nc = bacc.Bacc("TRN2", target_bir_lowering=False, debug=False,
                   enable_asserts=True, num_devices=N_CORES)

    qT_d = nc.dram_tensor("qT", [DQ, NQ], BF16, kind="ExternalInput").ap()
    kvT_d = nc.dram_tensor("kvT", [DKV, NK], BF16, kind="ExternalInput").ap()
    wq_d = nc.dram_tensor("Wq", [DQ, INNER], BF16, kind="ExternalInput").ap()
    wk_d = nc.dram_tensor("Wk", [DKV, INNER], BF16, kind="ExternalInput").ap()
    wv_d = nc.dram_tensor("Wv", [DKV, INNER], BF16, kind="ExternalInput").ap()
    wo_d = nc.dram_tensor("Wo", [INNER, DQ], BF16, kind="ExternalInput").ap()
    bo_d = nc.dram_tensor("bo", [DQ], F32, kind="ExternalInput").ap()
    out_d = nc.dram_tensor("out", [NQ, DQ], F32, kind="ExternalOutput").ap()

    MT_Q = DQ // 128      # 4
    MT_KV = DKV // 128    # 6
    IT = INNER // 128     # 4 inner tiles (= head pairs)
    KT = NK // 128        # 16
    QB = NQ // 512        # 2
    NT = NQ // 128        # 8 output row tiles
    PAIRS = HEADS // 2    # 4

    with tile.TileContext(nc) as tc:
        with (
            tc.tile_pool(name="consts", bufs=1) as consts,
            tc.tile_pool(name="wpool", bufs=1) as wpool,
            tc.tile_pool(name="xT", bufs=1) as xT_pool,
            tc.tile_pool(name="proj", bufs=1) as proj_pool,
            tc.tile_pool(name="attnT", bufs=1) as attnT_pool,
            tc.tile_pool(name="oacc", bufs=1) as oacc_pool,
            tc.tile_pool(name="exps", bufs=3) as exps_pool,
            tc.tile_pool(name="norm", bufs=1) as norm_pool,
            tc.tile_pool(name="outs", bufs=2) as outs_pool,
            tc.tile_pool(name="mm", bufs=2, space="PSUM") as ps_mm,
            tc.tile_pool(name="sc", bufs=2, space="PSUM") as ps_sc,
            tc.tile_pool(name="pv", bufs=2, space="PSUM") as ps_pv,
        ):
            wk_b = wpool.tile([128, MT_KV, 512], BF16, tag="wk")
            wq_b = wpool.tile([128, MT_Q, 512], BF16, tag="wq")
            wv_b = wpool.tile([128, MT_KV, 512], BF16, tag="wv")
            wo_b = wpool.tile([128, IT, 512], BF16, tag="wo")
            kvT = [xT_pool.tile([128, NK], BF16, tag=f"kvT{mt}", name=f"kvT{mt}")
                   for mt in range(MT_KV)]
            qT = [xT_pool.tile([128, NQ], BF16, tag=f"qT{mt}", name=f"qT{mt}")
                  for mt in range(MT_Q)]

            # ---- DMA issue order: the pre-attention critical prefix first
            # (kvT/qT columns 0:512 + wk/wq/wv), the rest streams in behind
            # the first attention block.
            nc.sync.dma_start(wk_b[:],
                              wk_d.rearrange("(t p) i -> p t i", p=128))
            nc.sync.dma_start(wq_b[:],
                              wq_d.rearrange("(t p) i -> p t i", p=128))
            for mt in range(MT_KV):
                nc.sync.dma_start(kvT[mt][:, 0:512],
                                  kvT_d[mt * 128:(mt + 1) * 128, 0:512])
            for mt in range(MT_Q):
                nc.sync.dma_start(qT[mt][:, 0:512],
                                  qT_d[mt * 128:(mt + 1) * 128, 0:512])
            nc.sync.dma_start(wv_b[:],
                              wv_d.rearrange("(t p) i -> p t i", p=128))
            for mt in range(MT_KV):
                nc.sync.dma_start(kvT[mt][:, 512:NK],
                                  kvT_d[mt * 128:(mt + 1) * 128, 512:NK])
            for mt in range(MT_Q):
                nc.sync.dma_start(qT[mt][:, 512:NQ],
                                  qT_d[mt * 128:(mt + 1) * 128, 512:NQ])
            nc.sync.dma_start(wo_b[:],
                              wo_d.rearrange("(t p) i -> p t i", p=128))

            # ---- constants ----
            ones1f = consts.tile([1, 64], F32)
            nc.vector.memset(ones1f[:], 1.0)
            ones1 = consts.tile([1, 64], F32R)
            nc.vector.tensor_copy(ones1[:], ones1f[:])
            ones8 = consts.tile([128, 8, 1], BF16)
            ones8f = consts.tile([128, 8, 1], F32)
            nc.vector.memset(ones8f[:], 1.0)
            nc.vector.tensor_copy(ones8[:], ones8f[:])
            warm = consts.tile([1, 64], mybir.dt.bfloat16)
            nc.scalar.activation(warm[:], ones1f[:], EXP, scale=0.125)
            bo_b = consts.tile([128, DQ], F32)
            nc.gpsimd.dma_start(
                out=bo_b[:],
                in_=bass.AP(tensor=bo_d.tensor, offset=bo_d.offset,
                            ap=[[0, 128]] + list(bo_d.ap)),
            )

            # ---- projection outputs / accumulators ----
            qhT = [proj_pool.tile([128, NQ], BF16, tag=f"qhT{i}", name=f"qhT{i}")
                   for i in range(IT)]
            khT = [proj_pool.tile([128, NK], BF16, tag=f"khT{i}", name=f"khT{i}")
                   for i in range(IT)]
            vh = [proj_pool.tile([128, HEADS, DA], BF16, tag=f"vh{k}", name=f"vh{k}")
                  for k in range(KT)]
            attnT = [attnT_pool.tile([128, NQ], BF16, tag=f"at{i}", name=f"at{i}")
                     for i in range(IT)]
            oacc = [oacc_pool.tile([128, DQ], F32, tag=f"oa{nt}", name=f"oa{nt}")
                    for nt in range(NT)]

            def emit_khT(it, nb):
                pp = ps_mm.tile([128, 512], F32, tag="mm", name="pp")
                for mt in range(MT_KV):
                    nc.tensor.matmul(
                        pp[:], wk_b[:, mt, it * 128:(it + 1) * 128],
                        kvT[mt][:, nb * 512:(nb + 1) * 512],
                        start=(mt == 0), stop=(mt == MT_KV - 1))
                nc.vector.tensor_copy(khT[it][:, nb * 512:(nb + 1) * 512], pp[:])

            def emit_qhT(it, nb):
                pp = ps_mm.tile([128, 512], F32, tag="mm", name="pp")
                for mt in range(MT_Q):
                    nc.tensor.matmul(
                        pp[:], wq_b[:, mt, it * 128:(it + 1) * 128],
                        qT[mt][:, nb * 512:(nb + 1) * 512],
                        start=(mt == 0), stop=(mt == MT_Q - 1))
                nc.vector.tensor_copy(qhT[it][:, nb * 512:(nb + 1) * 512], pp[:])

            def emit_vh(kt):
                pp = ps_mm.tile([128, 512], F32, tag="mm", name="pp")
                for mt in range(MT_KV):
                    nc.tensor.matmul(
                        pp[:], kvT[mt][:, kt * 128:(kt + 1) * 128],
                        wv_b[:, mt, :],
                        start=(mt == 0), stop=(mt == MT_KV - 1))
                nc.vector.tensor_copy(
                    vh[kt][:, :, 0:DH],
                    pp[:].rearrange("p (h d) -> p h d", h=HEADS))
                nc.vector.tensor_copy(vh[kt][:, :, DH:DA], ones8[:])

            def emit_opart(t, nt):
                # out-projection partial for head pair t, row tile nt:
                # oacc[nt] (+)= attnT[t][:, ns].T @ Wo[pair t rows]
                ns = slice(nt * 128, (nt + 1) * 128)
                po = ps_mm.tile([128, 512], F32, tag="mm", name="po")
                nc.tensor.matmul(po[:], attnT[t][:, ns], wo_b[:, t, :],
                                 start=True, stop=True)
                if t == 0:
                    nc.vector.tensor_add(oacc[nt][:], po[:], bo_b[:])
                else:
                    nc.vector.tensor_add(oacc[nt][:], po[:], oacc[nt][:])

            def emit_final(nt):
                # last out-projection partial (pair 3) + bias-carrying
                # accumulator -> output row tile store
                ns = slice(nt * 128, (nt + 1) * 128)
                po = ps_mm.tile([128, 512], F32, tag="mm", name="po")
                nc.tensor.matmul(po[:], attnT[3][:, ns], wo_b[:, 3, :],
                                 start=True, stop=True)
                ot = outs_pool.tile([128, DQ], F32, tag="ot", name="ot")
                nc.vector.tensor_add(ot[:], po[:], oacc[nt][:])
                nc.sync.dma_start(out_d[ns, :], ot[:])

            # pre-attention minimum: pair-0 projections for the first 512 q
            # and first 4 k-tiles; everything else rides as filler.
            emit_khT(0, 0)
            emit_qhT(0, 0)

            # PE filler queues per (pair, q-block). khT(0, nb) is consumed by
            # scores(kt=4nb..) of block (0, 0); vh(kt) by PV(kt) two
            # iterations after scores(kt); opart(t, nt) needs norm(t, nt//4)
            # which runs at the start of the next block after that.
            fillers = {
                (0, 0): ([(lambda kt=kt: emit_vh(kt)) for kt in (0, 1, 2)]
                         + [lambda: emit_khT(0, 1)]
                         + [(lambda kt=kt: emit_vh(kt)) for kt in (3, 4)]
                         + [lambda: emit_khT(0, 2)]
                         + [(lambda kt=kt: emit_vh(kt)) for kt in (5, 6)]
                         + [lambda: emit_khT(0, 3)]
                         + [lambda: emit_qhT(0, 1)]
                         + [(lambda kt=kt: emit_vh(kt)) for kt in range(7, KT)]),
                (0, 1): ([(lambda nb=nb: emit_khT(1, nb)) for nb in range(4)]
                         + [(lambda nb=nb: emit_qhT(1, nb)) for nb in range(QB)]),
                (1, 0): ([(lambda nt=nt: emit_opart(0, nt)) for nt in range(4)]
                         + [(lambda nb=nb: emit_khT(2, nb)) for nb in range(2)]),
                (1, 1): ([(lambda nb=nb: emit_khT(2, nb)) for nb in range(2, 4)]
                         + [(lambda nb=nb: emit_qhT(2, nb)) for nb in range(QB)]
                         + [(lambda nt=nt: emit_opart(0, nt)) for nt in range(4, 6)]),
                (2, 0): ([(lambda nt=nt: emit_opart(0, nt)) for nt in range(6, 8)]
                         + [(lambda nb=nb: emit_khT(3, nb)) for nb in range(2)]
                         + [(lambda nt=nt: emit_opart(1, nt)) for nt in range(2)]),
                (2, 1): ([(lambda nb=nb: emit_khT(3, nb)) for nb in range(2, 4)]
                         + [(lambda nb=nb: emit_qhT(3, nb)) for nb in range(QB)]
                         + [(lambda nt=nt: emit_opart(1, nt)) for nt in range(2, 4)]),
                (3, 0): ([(lambda nt=nt: emit_opart(1, nt)) for nt in range(4, 8)]
                         + [(lambda nt=nt: emit_opart(2, nt)) for nt in range(2)]),
                (3, 1): ([(lambda nt=nt: emit_opart(2, nt)) for nt in range(2, 8)]
                         + [(lambda nt=nt: emit_final(nt)) for nt in range(4)]),
            }

            carry = [None]

            def make_norm(t, qb, pvA, pvB):
                def emit():
                    qs = slice(qb * 512, (qb + 1) * 512)
                    dsb = norm_pool.tile([1, 1024], F32R, tag="nrm", name="dsb")
                    nc.vector.tensor_copy(dsb[0:1, 0:512], pvA[DH:DA, :])
                    nc.vector.tensor_copy(dsb[0:1, 512:1024], pvB[DH:DA, :])
                    dba = ps_mm.tile([64, 512], F32, tag="mm", name="dba")
                    dbb = ps_mm.tile([64, 512], F32, tag="mm", name="dbb")
                    rb = norm_pool.tile([64, 1024], F32, tag="nrm", name="rb")
                    nc.tensor.matmul(dba[:], ones1[:], dsb[0:1, 0:512],
                                     start=True, stop=True)
                    nc.vector.reciprocal_approx_fast(rb[:, 0:512], dba[:])
                    nc.vector.tensor_mul(attnT[t][0:64, qs],
                                         pvA[0:DH, :], rb[:, 0:512])
                    nc.tensor.matmul(dbb[:], ones1[:], dsb[0:1, 512:1024],
                                     start=True, stop=True)
                    nc.vector.reciprocal_approx_fast(rb[:, 512:1024], dbb[:])
                    nc.vector.tensor_mul(attnT[t][64:128, qs],
                                         pvB[0:DH, :], rb[:, 512:1024])
                return emit

            def norm_half_pre(t, qb, pvA, pvB, h):
                cs = slice(h * 256, (h + 1) * 256)
                dsb = norm_pool.tile([1, 512], F32R, tag="nrmd", name="dsb",
                                     bufs=2)
                nc.vector.tensor_copy(dsb[0:1, 0:256], pvA[DH:DA, cs])
                nc.vector.tensor_copy(dsb[0:1, 256:512], pvB[DH:DA, cs])
                return dsb

            def norm_half(t, qb, pvA, pvB, h, dsb):
                # normalize a 256-col half of the last block so the final
                # out-projection rows pipeline with the remaining half
                cs = slice(h * 256, (h + 1) * 256)
                qs = slice(qb * 512 + h * 256, qb * 512 + (h + 1) * 256)
                dba = ps_mm.tile([64, 256], F32, tag="mm", name="dba")
                dbb = ps_mm.tile([64, 256], F32, tag="mm", name="dbb")
                nc.tensor.matmul(dba[:], ones1[:], dsb[0:1, 0:256],
                                 start=True, stop=True)
                nc.tensor.matmul(dbb[:], ones1[:], dsb[0:1, 256:512],
                                 start=True, stop=True)
                rb = norm_pool.tile([64, 512], F32, tag="nrmh", name="rb")
                nc.vector.reciprocal_approx_fast(rb[:, 0:256], dba[:])
                nc.vector.reciprocal_approx_fast(rb[:, 256:512], dbb[:])
                nc.vector.tensor_mul(attnT[t][0:64, qs], pvA[0:DH, cs],
                                     rb[:, 0:256])
                nc.vector.tensor_mul(attnT[t][64:128, qs], pvB[0:DH, cs],
                                     rb[:, 256:512])

            # ---- attention, software-pipelined [scores(k+1) | filler | PV(k)]
            for t in range(PAIRS):
                hA, hB = 2 * t, 2 * t + 1
                for qb in range(QB):
                    qs = slice(qb * 512, (qb + 1) * 512)
                    todo = fillers[(t, qb)]
                    fi = 0

                    def emit_scores(kt):
                        ks = slice(kt * 128, (kt + 1) * 128)
                        sc = ps_sc.tile([128, 1024], F32, tag="sc", name="sc")
                        nc.tensor.matmul(
                            sc[:, 0:512],
                            khT[t][0:64, ks], qhT[t][0:64, qs],
                            start=True, stop=True, tile_position=(0, 0))
                        nc.tensor.matmul(
                            sc[:, 512:1024],
                            khT[t][64:128, ks], qhT[t][64:128, qs],
                            start=True, stop=True, tile_position=(64, 0))
                        ex = exps_pool.tile([128, 1024], BF16, tag="exp", name="ex")
                        nc.scalar.activation(ex[:], sc[:], EXP,
                                             scale=float(DH) ** -0.5)
                        return ex

                    # depth-2 software pipeline: PV(k) trails scores(k) by
                    # two iterations; the last two PVs + normalization of this
                    # block carry into the next block's prologue so ScalarE is
                    # never starved at block boundaries.
                    exq = [emit_scores(0)]
                    if carry[0]:
                        carry[0][0]()      # PV(14) of the previous block
                    exq.append(emit_scores(1))
                    if carry[0]:
                        carry[0][1]()      # PV(15) of the previous block
                        carry[0][2]()      # normalization (frees old pv tiles)
                        carry[0] = None
                    pvA = ps_pv.tile([DA, 512], F32, tag="pv", name="pvA")
                    pvB = ps_pv.tile([DA, 512], F32, tag="pv", name="pvB")

                    def mk_pv(kt, ex, pvA=pvA, pvB=pvB, hA=hA, hB=hB):
                        def emit():
                            nc.tensor.matmul(pvA[:], vh[kt][:, hA, :],
                                             ex[:, 0:512],
                                             start=(kt == 0), stop=(kt == KT - 1))
                            nc.tensor.matmul(pvB[:], vh[kt][:, hB, :],
                                             ex[:, 512:1024],
                                             start=(kt == 0), stop=(kt == KT - 1))
                        return emit

                    per_iter = 2 if (t, qb) in ((0, 0), (3, 1)) else 1
                    for kt in range(2, KT):
                        exq.append(emit_scores(kt))
                        for _ in range(per_iter):
                            if fi < len(todo) and (per_iter == 2 or kt % 2 == 0):
                                todo[fi]()
                                fi += 1
                        mk_pv(kt - 2, exq[kt - 2])()
                    mk_pv(KT - 2, exq[KT - 2])()
                    while fi < len(todo):
                        todo[fi]()
                        fi += 1
                    carry[0] = [mk_pv(KT - 1, exq[KT - 1]), lambda: None,
                                make_norm(t, qb, pvA, pvB), (pvA, pvB)]
            carry[0][0]()

            # ---- chunked tail: half-norms interleaved with final rows ----
            pvA_l, pvB_l = carry[0][3]
            dsb0 = norm_half_pre(3, 1, pvA_l, pvB_l, 0)
            dsb1 = norm_half_pre(3, 1, pvA_l, pvB_l, 1)
            norm_half(3, 1, pvA_l, pvB_l, 0, dsb0)
            emit_final(4)
            emit_final(5)
            norm_half(3, 1, pvA_l, pvB_l, 1, dsb1)
            emit_final(6)
            emit_final(7)

    nc.compile()
    return nc


def make_in_maps(q, kv, Wq, Wk, Wv, Wo, bo):
    """Host-side prep: bf16 casts + [dim, seq] transposes, per core."""
    import ml_dtypes
    bf16 = ml_dtypes.bfloat16

    wq_b = np.ascontiguousarray(np.asarray(Wq, np.float32).astype(bf16))
    wk_b = np.ascontiguousarray(np.asarray(Wk, np.float32).astype(bf16))
    wv_b = np.ascontiguousarray(np.asarray(Wv, np.float32).astype(bf16))
    wo_b = np.ascontiguousarray(np.asarray(Wo, np.float32).astype(bf16))
    bo_f = np.ascontiguousarray(np.asarray(bo, np.float32))
    q = np.asarray(q, np.float32)
    kv = np.asarray(kv, np.float32)

    kvT_b = [np.ascontiguousarray(kv[b].T.astype(bf16)) for b in range(B)]
    in_maps = []
    for c in range(N_CORES):
        b, h = c // 2, c % 2
        qT_b = np.ascontiguousarray(
            q[b, h * NQ:(h + 1) * NQ].T.astype(bf16))
        in_maps.append({
            "qT": qT_b, "kvT": kvT_b[b],
            "Wq": wq_b, "Wk": wk_b, "Wv": wv_b, "Wo": wo_b, "bo": bo_f,
        })
    return in_maps


def kernel(q, kv, Wq, Wk, Wv, Wo, bo):
    from concourse.bass_utils import run_bass_kernel_spmd

    if "nc" not in _cache:
        _cache["nc"] = _build()
    nc = _cache["nc"]

    in_maps = make_in_maps(q, kv, Wq, Wk, Wv, Wo, bo)
    res = run_bass_kernel_spmd(nc, in_maps, core_ids=list(range(N_CORES)))
    out = np.empty((B, NQ_FULL, DQ), dtype=np.float32)
    for c in range(N_CORES):
        b, h = c // 2, c % 2
        out[b, h * NQ:(h + 1) * NQ] = res.results[c]["out"]
    return out
